# revision 17
# baseline (speedup 1.0000x reference)
"""DiscourseGNN (GAT-style, dense-softmax) Trainium2 kernel, 8-core SPMD.

Math: the reference builds a dense (N,N) alpha with zeros at non-edges and
softmaxes full rows.  Scores are O(1) here, so no max-subtraction is needed
and softmax is computed analytically from the sparse part:
  s_ij = exp(lrelu(e_ij)) at edges, exp(0)=1 elsewhere
  h_new_i = (S_tot + U_i) / (N + d_i)
    U_i = sum_{j in N(i)} c_ij * hw_j,  c_ij = exp(lrelu(p_i + q_j)) - 1
    d_i = sum_j c_ij,  S_tot = colsum(hw)  (kept fp32-exact)
    p = hw @ a[:H],  q = hw @ a[H:]
The sparse correction is ~1e-4 of h, so it runs in fp8; everything dense
(projections, S_tot, LN, elu) is fp32.

Sharding: rows (= src = softmax rows) split 1024/core; edge lists partitioned
by src; hw plus per-node (p, q) AllGathered once per layer; the sparse matmul
runs on PE via 128-edge chunks: dma_gather of hw rows (fp8), DVE weighting by
c, and a static 0/1 lane->src indicator as the stationary operand.
"""

import sys

import numpy as np

if "/opt/trn_rl_repo" not in sys.path:
    sys.path.insert(0, "/opt/trn_rl_repo")

import ml_dtypes  # noqa: E402

import concourse.bacc as bacc  # noqa: E402
import concourse.bass as bass  # noqa: E402
import concourse.mybir as mybir  # noqa: E402
import concourse.tile as tile  # noqa: E402
from concourse import library_config  # noqa: E402

N, DIN, H, E, L = 8192, 512, 256, 262144, 2
NCORES = 8
NL = N // NCORES
NBLK = NL // 128
LN_EPS = 1e-5
F32 = mybir.dt.float32
F8 = mybir.dt.float8e4
I16 = mybir.dt.int16
AF = mybir.ActivationFunctionType
ALU = mybir.AluOpType
FP8NP = ml_dtypes.float8_e4m3


# ================================================================ host prep
def _preprocess(edge_index):
    src = np.asarray(edge_index[0]).astype(np.int64)
    tgt = np.asarray(edge_index[1]).astype(np.int64)
    per_core, ncb = [], 1
    for c in range(NCORES):
        sel = (src >= c * NL) & (src < (c + 1) * NL)
        s, t = src[sel] - c * NL, tgt[sel]
        o = np.lexsort((t, s))
        s, t = s[o], t[o]
        blocks = []
        for b in range(NBLK):
            m = (s // 128) == b
            blocks.append((s[m], t[m]))
            ncb = max(ncb, (len(s[m]) + 127) // 128)
        per_core.append(blocks)
    return per_core, ncb


def _wrap16(arr):
    """list[i] -> out[16g + i%16, i//16] replicated for all 8 groups g."""
    n = len(arr)
    a = np.asarray(arr, np.int16).reshape(n // 16, 16).T
    return np.tile(a, (8, 1))


def _build_core_arrays(blocks, ncb):
    ncht = NBLK * ncb
    nstream = 16 * ncht
    gidx = np.zeros((128, ncht * 8), np.int16)
    ind = np.zeros((128, ncht, 128), FP8NP)
    srcl = np.zeros((128, ncht), np.int64)
    tgtl = np.zeros((128, ncht), np.int64)
    real = np.zeros((128, ncht), bool)
    for b in range(NBLK):
        s, t = blocks[b]
        cnt = len(s)
        glist = np.zeros(ncb * 128, np.int64)
        glist[:cnt] = t
        gidx[:, b * ncb * 8:(b + 1) * ncb * 8] = _wrap16(glist)
        for k in range(ncb):
            col = b * ncb + k
            lo = k * 128
            nh = max(0, min(128, cnt - lo))
            if nh:
                ind[np.arange(nh), col, s[lo:lo + nh] - b * 128] = 1.0
                srcl[:nh, col] = s[lo:lo + nh]
                tgtl[:nh, col] = t[lo:lo + nh]
                real[:nh, col] = True
    # ap_gather group lists: group g, list pos i = r*ncht + col for edge
    # (lane 16g+r, chunk col).  idx storage is wrapped: [16g + i%16, i//16].
    pidx = np.zeros((128, ncht), np.int16)
    qidx = np.zeros((128, ncht), np.int16)
    pmask = np.zeros((128, nstream), FP8NP)
    for g in range(8):
        lanes = 16 * g + np.arange(16)
        plist = srcl[lanes, :].reshape(nstream)   # i = r*ncht + col
        qlist = tgtl[lanes, :].reshape(nstream)
        mlist = real[lanes, :].reshape(nstream).astype(np.float32)
        pidx[16 * g:16 * g + 16] = plist.reshape(nstream // 16, 16).T
        qidx[16 * g:16 * g + 16] = qlist.reshape(nstream // 16, 16).T
        pmask[16 * g:16 * g + 16] = mlist[None, :].astype(FP8NP)
    return gidx, ind.reshape(128, ncht * 128), pmask, pidx, qidx


# ================================================================ device prog
def _emit(nc, tc, ncb, io):
    ncht = NBLK * ncb
    nstream = 16 * ncht
    nhalf = nstream // 2
    v, sc, gp, te = nc.vector, nc.scalar, nc.gpsimd, nc.tensor
    sy = nc.sync
    gp.load_library(library_config.ap_gather)
    rg = [list(range(NCORES))]

    with tc.tile_pool(name="dram", bufs=1, space="DRAM") as dram, \
         tc.tile_pool(name="sb", bufs=1) as sb:
        # ---- persistent SBUF
        h_all = sb.tile([128, NBLK, H], F32)
        ht = sb.tile([128, 2, NL], F32)
        hwpq = sb.tile([128, NBLK, H + 2], F32)
        tblin = sb.tile([128, NBLK, H], F8)
        ident = sb.tile([128, 128], F32)
        ones128 = sb.tile([128, 1], F32)
        ones_row = sb.tile([1, 128], F32)
        ones8 = sb.tile([8, 1], F32)
        ind_sb = sb.tile([128, ncht, 128], F8)
        gidx_sb = sb.tile([128, ncht * 8], I16)
        pidx_sb = sb.tile([128, ncht], I16)
        qidx_sb = sb.tile([128, ncht], I16)
        pmask_sb = sb.tile([128, nstream], F8)
        q_bcast = sb.tile([128, N], F32)
        p_bcast = sb.tile([128, NL], F32)
        c_all = sb.tile([128, ncht], F32)
        c8 = sb.tile([128, ncht], F8)
        wg_sb = sb.tile([128, 2, H], F32)
        wgt_sb = sb.tile([128, 2, H], F32)
        rhs_l = sb.tile([128, 2, H + 2], F32)
        a_cols = sb.tile([128, 4], F32)
        a1p = sb.tile([128, 2], F32)
        a2p = sb.tile([128, 2], F32)
        prow_t = sb.tile([1, 1024], F32)
        qrow_t = sb.tile([1, 1024], F32)
        srow_t = sb.tile([1, 1024], F32)
        stt = sb.tile([8, H + 2], F32)
        s_bcast = sb.tile([128, H], F32)
        srow = sb.tile([1, H + 2], F32)
        bin_b = sb.tile([128, H], F32)
        g_b = sb.tile([128, H], F32)
        b_b = sb.tile([128, H], F32)
        wpool_b = sb.tile([128, H], F32)
        bpb = sb.tile([128, 1], F32)
        cstr_a = sb.tile([128, nhalf], F32)
        cstr_b = sb.tile([128, nhalf], F32)
        tmp_r = sb.tile([1, H], F32)
        tmp_1 = sb.tile([1, 1], F32)
        winT = sb.tile([128, 4, H], F32)

        bridge = dram.tile([128 * nstream], F32)

        v.memset(ones128[:], 1.0)
        v.memset(ones_row[:], 1.0)
        v.memset(ones8[:], 1.0)
        sy.dma_start(ident[:], io["ident"][:])
        sy.dma_start(ind_sb[:], io["ind"].rearrange("p (a b) -> p a b", b=128))
        sy.dma_start(gidx_sb[:], io["gidx"][:])
        sy.dma_start(pidx_sb[:], io["pidx"][:])
        sy.dma_start(qidx_sb[:], io["qidx"][:])
        sy.dma_start(pmask_sb[:], io["pmask"][:])

        _bc_ctx = tc.tile_pool(name="bc", bufs=2, space="PSUM")
        bcpool = _bc_ctx.__enter__()

        def bcast(dst, src_row):
            X = src_row.shape[-1]
            assert dst.shape[0] == 128 and dst.shape[-1] == X
            for lo in range(0, X, 512):
                w = min(512, X - lo)
                ps = bcpool.tile([128, 512], F32, tag="bc", name="bc_ps")
                te.matmul(ps[:, 0:w], ones_row[:], src_row[:, lo:lo + w],
                          start=True, stop=True)
                v.tensor_copy(dst[:, lo:lo + w], ps[:, 0:w])

        sy.dma_start(tmp_r[:], io["b_in"][None, :])
        bcast(bin_b, tmp_r[:])
        sy.dma_start(tmp_r[:], io["W_pool"][:])
        bcast(wpool_b, tmp_r[:])
        sy.dma_start(tmp_1[:], io["b_pool"][None, :])
        bcast(bpb, tmp_1[:])

        # ---- input projection h = nf @ W_in^T + b_in
        with tc.tile_pool(name="tp0", bufs=2, space="PSUM") as tpool, \
             tc.tile_pool(name="pp0", bufs=2, space="PSUM") as ppool, \
             tc.tile_pool(name="nf0", bufs=2) as nfpool:
            win_sb = nfpool.tile([128, 2, DIN], F32, tag="win", bufs=1)
            sy.dma_start(win_sb[:], io["W_in"].rearrange("(a p) d -> p a d", p=128))
            for kd in range(4):
                for ko in range(2):
                    pt = tpool.tile([128, 128], F32, tag="pt")
                    te.transpose(pt[:], win_sb[:, ko, kd * 128:(kd + 1) * 128],
                                 ident[:])
                    v.tensor_copy(winT[:, kd, ko * 128:(ko + 1) * 128], pt[:])
            for m in range(NBLK):
                nf_m = nfpool.tile([128, DIN], F32, tag="nf")
                sy.dma_start(nf_m[:], io["nf"][m * 128:(m + 1) * 128, :])
                nfT_m = nfpool.tile([128, 4, 128], F32, tag="nft")
                for kd in range(4):
                    pt = tpool.tile([128, 128], F32, tag="pt")
                    te.transpose(pt[:], nf_m[:, kd * 128:(kd + 1) * 128], ident[:])
                    v.tensor_copy(nfT_m[:, kd, :], pt[:])
                ph = ppool.tile([128, H], F32, tag="ph")
                for kd in range(4):
                    te.matmul(ph[:], nfT_m[:, kd, :], winT[:, kd, :],
                              start=(kd == 0), stop=(kd == 3))
                v.tensor_tensor(h_all[:, m, :], ph[:], bin_b[:], op=ALU.add)

        # ---- layers
        for l in range(L):
            bounce1 = dram.tile([NL, H], F8, name=f"bounce1_{l}")
            table = dram.tile([N, H], F8, name=f"table_{l}")
            bounce2 = dram.tile([3, 1024], F32, name=f"bounce2_{l}")
            ag2out = dram.tile([24, 1024], F32, addr_space="Shared",
                               name=f"ag2out_{l}")
            sy.dma_start(wg_sb[:], io["W_gat"][l].rearrange("(a p) d -> p a d",
                                                            p=128))
            sy.dma_start(a_cols[:], io["a_gat"][l].rearrange("(x p) -> p x", p=128))
            sy.dma_start(tmp_r[:], io["ln_g"][l][None, :])
            bcast(g_b, tmp_r[:])
            sy.dma_start(tmp_r[:], io["ln_b"][l][None, :])
            bcast(b_b, tmp_r[:])

            with tc.tile_pool(name=f"tp{l}", bufs=2, space="PSUM") as tpool, \
                 tc.tile_pool(name=f"pp{l}", bufs=3, space="PSUM") as ppool:
                for m in range(NBLK):
                    for kf in range(2):
                        pt = tpool.tile([128, 128], F32, tag="pt")
                        te.transpose(pt[:], h_all[:, m, kf * 128:(kf + 1) * 128],
                                     ident[:])
                        v.tensor_copy(ht[:, kf, m * 128:(m + 1) * 128], pt[:])
                for kf in range(2):
                    for ko in range(2):
                        pt = tpool.tile([128, 128], F32, tag="pt")
                        te.transpose(pt[:], wg_sb[:, ko, kf * 128:(kf + 1) * 128],
                                     ident[:])
                        v.tensor_copy(wgt_sb[:, kf, ko * 128:(ko + 1) * 128], pt[:])
                for it in range(2):
                    p1 = ppool.tile([128, 1], F32, tag="pp")
                    for ko in range(2):
                        te.matmul(p1[:], wg_sb[:, ko, it * 128:(it + 1) * 128],
                                  a_cols[:, ko:ko + 1], start=(ko == 0),
                                  stop=(ko == 1))
                    v.tensor_copy(a1p[:, it:it + 1], p1[:])
                    p2 = ppool.tile([128, 1], F32, tag="pp")
                    for ko in range(2):
                        te.matmul(p2[:], wg_sb[:, ko, it * 128:(it + 1) * 128],
                                  a_cols[:, 2 + ko:3 + ko], start=(ko == 0),
                                  stop=(ko == 1))
                    v.tensor_copy(a2p[:, it:it + 1], p2[:])
                for kf in range(2):
                    v.tensor_copy(rhs_l[:, kf, 0:H], wgt_sb[:, kf, :])
                    v.tensor_copy(rhs_l[:, kf, H:H + 1], a1p[:, kf:kf + 1])
                    v.tensor_copy(rhs_l[:, kf, H + 1:H + 2], a2p[:, kf:kf + 1])
                for m in range(NBLK):
                    ph = ppool.tile([128, H + 2], F32, tag="pp")
                    for kf in range(2):
                        te.matmul(ph[:], ht[:, kf, m * 128:(m + 1) * 128],
                                  rhs_l[:, kf, :], start=(kf == 0), stop=(kf == 1))
                    v.tensor_copy(hwpq[:, m, :], ph[:])
                    v.tensor_copy(tblin[:, m, :], ph[:, 0:H])
                v.memset(srow_t[:], 0.0)
                for half in range(2):
                    pp1 = ppool.tile([1, 512], F32, tag="pp")
                    for kf in range(2):
                        te.matmul(pp1[:], a1p[:, kf:kf + 1],
                                  ht[:, kf, half * 512:(half + 1) * 512],
                                  start=(kf == 0), stop=(kf == 1))
                    v.tensor_copy(prow_t[:, half * 512:(half + 1) * 512], pp1[:])
                    pq1 = ppool.tile([1, 512], F32, tag="pp")
                    for kf in range(2):
                        te.matmul(pq1[:], a2p[:, kf:kf + 1],
                                  ht[:, kf, half * 512:(half + 1) * 512],
                                  start=(kf == 0), stop=(kf == 1))
                    v.tensor_copy(qrow_t[:, half * 512:(half + 1) * 512], pq1[:])
                ps = ppool.tile([1, H + 2], F32, tag="pp")
                for m in range(NBLK):
                    te.matmul(ps[:], ones128[:], hwpq[:, m, :],
                              start=(m == 0), stop=(m == NBLK - 1))
                v.tensor_copy(srow_t[:, 0:H + 2], ps[:])

                sy.dma_start(bounce2[0:1, :], prow_t[:])
                sy.dma_start(bounce2[1:2, :], qrow_t[:])
                sy.dma_start(bounce2[2:3, :], srow_t[:])
                gp.collective_compute("AllGather", ALU.bypass, replica_groups=rg,
                                      ins=[bounce2[:].opt()], outs=[ag2out[:].opt()])
                sy.dma_start(bounce1[:].rearrange("(a p) d -> p a d", p=128),
                             tblin[:])
                gp.collective_compute("AllGather", ALU.bypass, replica_groups=rg,
                                      ins=[bounce1[:].opt()], outs=[table[:].opt()])

                ag2v = ag2out[:].rearrange("(r x) d -> r x d", x=3)
                sy.dma_start(stt[:], ag2v[:, 2, 0:H + 2])
                pst = ppool.tile([1, H + 2], F32, tag="pp")
                te.matmul(pst[:], ones8[:], stt[:], start=True, stop=True)
                v.tensor_copy(srow[:], pst[:])
                bcast(s_bcast, srow[:, 0:H])

                # q table: stage 8 q-rows into partition 0 then broadcast
                src_q = bass.AP(ag2out[:].tensor, 1024,
                                [[0, 1], [3072, 8], [1, 1024]])
                sy.dma_start(q_bcast[0:1, :].rearrange("p (r d) -> p r d", r=8),
                             src_q)
                bcast(q_bcast, q_bcast[0:1, :])
                bcast(p_bcast, prow_t[:])

                # per-edge coefficients, two half-stream passes
                for hf in range(2):
                    i0 = hf * nhalf
                    idx_sl = slice(hf * (ncht // 2), (hf + 1) * (ncht // 2))
                    gp.ap_gather(cstr_a[:], p_bcast[:], pidx_sb[:, idx_sl],
                                 128, NL, 1, nhalf)
                    gp.ap_gather(cstr_b[:], q_bcast[:], qidx_sb[:, idx_sl],
                                 128, N, 1, nhalf)
                    v.tensor_tensor(cstr_a[:], cstr_a[:], cstr_b[:], op=ALU.add)
                    v.tensor_scalar(cstr_b[:], cstr_a[:], 0.2, None, op0=ALU.mult)
                    v.tensor_tensor(cstr_a[:], cstr_a[:], cstr_b[:], op=ALU.max)
                    sc.activation(cstr_a[:], cstr_a[:], AF.Exp)
                    v.tensor_scalar(cstr_a[:], cstr_a[:], -1.0, None, op0=ALU.add)
                    v.tensor_tensor(cstr_a[:], cstr_a[:],
                                    pmask_sb[:, i0:i0 + nhalf], op=ALU.mult)
                    sy.dma_start(
                        bridge[:].rearrange("(p i) -> p i", p=128)[:, i0:i0 + nhalf],
                        cstr_a[:])
                # regroup: flat pos (16g+r)*nstream + r*ncht + col -> [p, col]
                src_ap = bass.AP(bridge[:].tensor, 0,
                                 [[16 * nstream, 8], [nstream + ncht, 16],
                                  [1, ncht]])
                sy.dma_start(c_all[:], src_ap)
                v.tensor_copy(c8[:], c_all[:])

            with tc.tile_pool(name=f"sp{l}", bufs=3, space="PSUM") as spool, \
                 tc.tile_pool(name=f"gp{l}", bufs=6) as gpool, \
                 tc.tile_pool(name=f"ep{l}", bufs=3) as ep:
                nreg = {}
                for k0 in range(0, ncb, 4):
                    kw = min(4, ncb - k0)
                    if kw * 128 not in nreg:
                        nreg[kw * 128] = gp.to_reg(kw * 128)
                for b in range(NBLK):
                    pb = spool.tile([128, H + 1], F32, tag="pb")
                    for k0 in range(0, ncb, 4):
                        kw = min(4, ncb - k0)
                        col = b * ncb + k0
                        g1 = gpool.tile([128, 4, H], F8, tag="g1")
                        gp.dma_gather(
                            g1[:, 0:kw, :], table[:],
                            gidx_sb[:, col * 8:(col + kw) * 8],
                            kw * 128, nreg[kw * 128], H)
                        wt = gpool.tile([128, 4, H + 1], F8, tag="wt")
                        csl = c8[:, col:col + kw]
                        v.tensor_tensor(wt[:, 0:kw, 0:H], g1[:, 0:kw, :],
                                        csl.unsqueeze(2).to_broadcast(
                                            [128, kw, H]), op=ALU.mult)
                        v.tensor_copy(wt[:, 0:kw, H], csl)
                        for k in range(kw):
                            te.matmul(pb[:], ind_sb[:, col + k, :],
                                      wt[:, k, :], start=(k0 + k == 0),
                                      stop=(k0 + k == ncb - 1))
                    denom = ep.tile([128, 1], F32, tag="den")
                    v.tensor_scalar(denom[:], pb[:, H:H + 1], float(N), None,
                                    op0=ALU.add)
                    v.reciprocal(denom[:], denom[:])
                    hn = ep.tile([128, H], F32, tag="hn")
                    v.tensor_tensor(hn[:], pb[:, 0:H], s_bcast[:], op=ALU.add)
                    v.tensor_scalar(hn[:], hn[:], denom[:], None, op0=ALU.mult)
                    v.tensor_tensor(hn[:], hn[:], h_all[:, b, :], op=ALU.add)
                    mu = ep.tile([128, 1], F32, tag="mu")
                    v.tensor_reduce(mu[:], hn[:], axis=mybir.AxisListType.X,
                                    op=ALU.add)
                    v.tensor_scalar(mu[:], mu[:], 1.0 / H, None, op0=ALU.mult)
                    v.tensor_scalar(hn[:], hn[:], mu[:], None, op0=ALU.subtract)
                    var = ep.tile([128, 1], F32, tag="var")
                    sq = ep.tile([128, H], F32, tag="sq")
                    sc.activation(sq[:], hn[:], AF.Square, accum_out=var[:])
                    v.tensor_scalar(var[:], var[:], 1.0 / H, None, op0=ALU.mult)
                    v.tensor_scalar(var[:], var[:], LN_EPS, None, op0=ALU.add)
                    sc.activation(var[:], var[:], AF.Sqrt)
                    v.reciprocal(var[:], var[:])
                    v.tensor_scalar(hn[:], hn[:], var[:], None, op0=ALU.mult)
                    v.tensor_tensor(hn[:], hn[:], g_b[:], op=ALU.mult)
                    v.tensor_tensor(hn[:], hn[:], b_b[:], op=ALU.add)
                    ex = ep.tile([128, H], F32, tag="ex")
                    sc.activation(ex[:], hn[:], AF.Exp)
                    v.tensor_scalar(ex[:], ex[:], -1.0, None, op0=ALU.add)
                    v.tensor_scalar(ex[:], ex[:], 0.0, None, op0=ALU.min)
                    v.tensor_tensor(h_all[:, b, :], hn[:], ex[:], op=ALU.max)

        # ---- readout
        with tc.tile_pool(name="fp", bufs=1, space="PSUM") as fpool, \
             tc.tile_pool(name="fe", bufs=3) as fe:
            pg = fpool.tile([1, H], F32, tag="pg")
            for b in range(NBLK):
                gl = fe.tile([128, H], F32, tag="gl")
                v.tensor_tensor(gl[:], h_all[:, b, :], wpool_b[:], op=ALU.mult)
                glr = fe.tile([128, 1], F32, tag="glr")
                v.tensor_reduce(glr[:], gl[:], axis=mybir.AxisListType.X,
                                op=ALU.add)
                v.tensor_tensor(glr[:], glr[:], bpb[:], op=ALU.add)
                sc.activation(glr[:], glr[:], AF.Sigmoid)
                gh = fe.tile([128, H], F32, tag="gh")
                v.tensor_scalar(gh[:], h_all[:, b, :], glr[:], None, op0=ALU.mult)
                te.matmul(pg[:], ones128[:], gh[:], start=(b == 0),
                          stop=(b == NBLK - 1))
            ge_sb = fe.tile([1, H], F32, tag="ge")
            v.tensor_copy(ge_sb[:], pg[:])
            sy.dma_start(io["ge_out"][:], ge_sb[:])
            sy.dma_start(io["h_out"].rearrange("(a p) d -> p a d", p=128), h_all[:])


def _declare_io(nc, ncb):
    ncht = NBLK * ncb
    io = {}

    def inp(name, shape, dt):
        io[name] = nc.dram_tensor(name, shape, dt, kind="ExternalInput").ap()

    inp("nf", [NL, DIN], F32)
    inp("W_in", [H, DIN], F32)
    inp("b_in", [H], F32)
    inp("W_gat", [L, H, H], F32)
    inp("a_gat", [L, 2 * H], F32)
    inp("ln_g", [L, H], F32)
    inp("ln_b", [L, H], F32)
    inp("W_pool", [1, H], F32)
    inp("b_pool", [1], F32)
    inp("ident", [128, 128], F32)
    inp("gidx", [128, ncht * 8], I16)
    inp("pidx", [128, ncht], I16)
    inp("qidx", [128, ncht], I16)
    inp("pmask", [128, 16 * ncht], F8)
    inp("ind", [128, ncht * 128], F8)
    io["h_out"] = nc.dram_tensor("h_out", [NL, H], F32, kind="ExternalOutput").ap()
    io["ge_out"] = nc.dram_tensor("ge_out", [1, H], F32, kind="ExternalOutput").ap()
    return io


def build(ncb):
    nc = bacc.Bacc("TRN2", target_bir_lowering=False, debug=False,
                   num_devices=NCORES)
    io = _declare_io(nc, ncb)
    with tile.TileContext(nc) as tc:
        _emit(nc, tc, ncb, io)
    nc.compile()
    return nc


def make_in_maps(inputs):
    per_core, ncb = _preprocess(inputs["edge_index"])
    nf = np.asarray(inputs["node_features"], np.float32)
    base = {k: np.ascontiguousarray(np.asarray(inputs[k], np.float32))
            for k in ("W_in", "b_in", "W_gat", "a_gat", "ln_g", "ln_b",
                      "W_pool", "b_pool")}
    base["ident"] = np.eye(128, dtype=np.float32)
    in_maps = []
    for c in range(NCORES):
        gidx, ind, pmask, pidx, qidx = _build_core_arrays(per_core[c], ncb)
        m = dict(base)
        m["nf"] = np.ascontiguousarray(nf[c * NL:(c + 1) * NL])
        m["gidx"], m["ind"], m["pmask"] = gidx, ind, pmask
        m["pidx"], m["qidx"] = pidx, qidx
        in_maps.append(m)
    return in_maps, ncb


def kernel(**inputs):
    from concourse.bass_utils import run_bass_kernel_spmd
    in_maps, ncb = make_in_maps(inputs)
    nc = build(ncb)
    res = run_bass_kernel_spmd(nc, in_maps, list(range(NCORES)))
    h = np.concatenate([r["h_out"] for r in res.results], axis=0)
    ge = np.stack([r["ge_out"][0] for r in res.results]).sum(0).astype(np.float32)
    return h, ge


# revision 18
# speedup vs baseline: 1.1028x; 1.1028x over previous
"""DiscourseGNN (GAT-style, dense-softmax) Trainium2 kernel, 8-core SPMD.

Math: the reference builds a dense (N,N) alpha with zeros at non-edges and
softmaxes full rows.  Scores are O(1) here, so no max-subtraction is needed
and softmax is computed analytically from the sparse part:
  s_ij = exp(lrelu(e_ij)) at edges, exp(0)=1 elsewhere
  h_new_i = (S_tot + U_i) / (N + d_i)
    U_i = sum_{j in N(i)} c_ij * hw_j,  c_ij = exp(lrelu(p_i + q_j)) - 1
    d_i = sum_j c_ij,  S_tot = colsum(hw)  (kept fp32-exact)
    p = hw @ a[:H],  q = hw @ a[H:]
The sparse correction is ~1e-4 of h, so it runs in fp8; everything dense
(projections, S_tot, LN, elu) is fp32.

Sharding: rows (= src = softmax rows) split 1024/core; edge lists partitioned
by src; hw plus per-node (p, q) AllGathered once per layer; the sparse matmul
runs on PE via 128-edge chunks: dma_gather of hw rows (fp8), DVE weighting by
c, and a static 0/1 lane->src indicator as the stationary operand.
"""

import sys

import numpy as np

if "/opt/trn_rl_repo" not in sys.path:
    sys.path.insert(0, "/opt/trn_rl_repo")

import ml_dtypes  # noqa: E402

import concourse.bacc as bacc  # noqa: E402
import concourse.bass as bass  # noqa: E402
import concourse.mybir as mybir  # noqa: E402
import concourse.tile as tile  # noqa: E402
from concourse import library_config  # noqa: E402

N, DIN, H, E, L = 8192, 512, 256, 262144, 2
NCORES = 8
NL = N // NCORES
NBLK = NL // 128
LN_EPS = 1e-5
F32 = mybir.dt.float32
F8 = mybir.dt.float8e4
I16 = mybir.dt.int16
AF = mybir.ActivationFunctionType
ALU = mybir.AluOpType
FP8NP = ml_dtypes.float8_e4m3


# ================================================================ host prep
def _preprocess(edge_index):
    src = np.asarray(edge_index[0]).astype(np.int64)
    tgt = np.asarray(edge_index[1]).astype(np.int64)
    per_core, ncb = [], 1
    for c in range(NCORES):
        sel = (src >= c * NL) & (src < (c + 1) * NL)
        s, t = src[sel] - c * NL, tgt[sel]
        o = np.lexsort((t, s))
        s, t = s[o], t[o]
        blocks = []
        for b in range(NBLK):
            m = (s // 128) == b
            blocks.append((s[m], t[m]))
            ncb = max(ncb, (len(s[m]) + 127) // 128)
        per_core.append(blocks)
    return per_core, ncb


def _wrap16(arr):
    """list[i] -> out[16g + i%16, i//16] replicated for all 8 groups g."""
    n = len(arr)
    a = np.asarray(arr, np.int16).reshape(n // 16, 16).T
    return np.tile(a, (8, 1))


def _build_core_arrays(blocks, ncb):
    ncht = NBLK * ncb
    nstream = 16 * ncht
    gidx = np.zeros((128, ncht * 8), np.int16)
    ind = np.zeros((128, ncht, 128), FP8NP)
    srcl = np.zeros((128, ncht), np.int64)
    tgtl = np.zeros((128, ncht), np.int64)
    real = np.zeros((128, ncht), bool)
    for b in range(NBLK):
        s, t = blocks[b]
        cnt = len(s)
        glist = np.zeros(ncb * 128, np.int64)
        glist[:cnt] = t
        gidx[:, b * ncb * 8:(b + 1) * ncb * 8] = _wrap16(glist)
        for k in range(ncb):
            col = b * ncb + k
            lo = k * 128
            nh = max(0, min(128, cnt - lo))
            if nh:
                ind[np.arange(nh), col, s[lo:lo + nh] - b * 128] = 1.0
                srcl[:nh, col] = s[lo:lo + nh]
                tgtl[:nh, col] = t[lo:lo + nh]
                real[:nh, col] = True
    # ap_gather group lists: group g, list pos i = r*ncht + col for edge
    # (lane 16g+r, chunk col).  idx storage is wrapped: [16g + i%16, i//16].
    pidx = np.zeros((128, ncht), np.int16)
    qidx = np.zeros((128, ncht), np.int16)
    pmask = np.zeros((128, nstream), FP8NP)
    for g in range(8):
        lanes = 16 * g + np.arange(16)
        plist = srcl[lanes, :].reshape(nstream)   # i = r*ncht + col
        qlist = tgtl[lanes, :].reshape(nstream)
        mlist = real[lanes, :].reshape(nstream).astype(np.float32)
        pidx[16 * g:16 * g + 16] = plist.reshape(nstream // 16, 16).T
        qidx[16 * g:16 * g + 16] = qlist.reshape(nstream // 16, 16).T
        pmask[16 * g:16 * g + 16] = mlist[None, :].astype(FP8NP)
    return gidx, ind.reshape(128, ncht * 128), pmask, pidx, qidx


# ================================================================ device prog
def _emit(nc, tc, ncb, io):
    ncht = NBLK * ncb
    nstream = 16 * ncht
    nhalf = nstream // 2
    v, sc, gp, te = nc.vector, nc.scalar, nc.gpsimd, nc.tensor
    sy = nc.sync
    gp.load_library(library_config.ap_gather)
    rg = [list(range(NCORES))]

    with tc.tile_pool(name="dram", bufs=1, space="DRAM") as dram, \
         tc.tile_pool(name="sb", bufs=1) as sb:
        # ---- persistent SBUF
        h_all = sb.tile([128, NBLK, H], F32)
        ht = sb.tile([128, 2, NL], F32)
        hwpq = sb.tile([128, NBLK, H + 2], F32)
        tblin = sb.tile([128, NBLK, H], F8)
        ident = sb.tile([128, 128], F32)
        ones128 = sb.tile([128, 1], F32)
        ones_row = sb.tile([1, 128], F32)
        ones8 = sb.tile([8, 1], F32)
        ind_sb = sb.tile([128, ncht, 128], F8)
        gidx_sb = sb.tile([128, ncht * 8], I16)
        pidx_sb = sb.tile([128, ncht], I16)
        qidx_sb = sb.tile([128, ncht], I16)
        pmask_sb = sb.tile([128, nstream], F8)
        q_bcast = sb.tile([128, N], F32)
        p_bcast = sb.tile([128, NL], F32)
        c_all = sb.tile([128, ncht], F32)
        c8 = sb.tile([128, ncht], F8)
        wg_sb = sb.tile([128, 2, H], F32)
        wgt_sb = sb.tile([128, 2, H], F32)
        rhs_l = sb.tile([128, 2, H + 2], F32)
        a_cols = sb.tile([128, 4], F32)
        a1p = sb.tile([128, 2], F32)
        a2p = sb.tile([128, 2], F32)
        prow_t = sb.tile([1, 1024], F32)
        qrow_t = sb.tile([1, 1024], F32)
        srow_t = sb.tile([1, 1024], F32)
        stt = sb.tile([8, H + 2], F32)
        s_bcast = sb.tile([128, H], F32)
        srow = sb.tile([1, H + 2], F32)
        bin_b = sb.tile([128, H], F32)
        g_b = sb.tile([128, H], F32)
        b_b = sb.tile([128, H], F32)
        wpool_b = sb.tile([128, H], F32)
        bpb = sb.tile([128, 1], F32)
        cstr_a = sb.tile([128, nhalf], F32)
        cstr_b = sb.tile([128, nhalf], F32)
        tmp_r = sb.tile([1, H], F32)
        tmp_1 = sb.tile([1, 1], F32)
        winT = sb.tile([128, 4, H], F32)

        bridge = dram.tile([128 * nstream], F32)

        v.memset(ones128[:], 1.0)
        v.memset(ones_row[:], 1.0)
        v.memset(ones8[:], 1.0)
        sy.dma_start(ident[:], io["ident"][:])
        sy.dma_start(ind_sb[:], io["ind"].rearrange("p (a b) -> p a b", b=128))
        sy.dma_start(gidx_sb[:], io["gidx"][:])
        sy.dma_start(pidx_sb[:], io["pidx"][:])
        sy.dma_start(qidx_sb[:], io["qidx"][:])
        sy.dma_start(pmask_sb[:], io["pmask"][:])

        _bc_ctx = tc.tile_pool(name="bc", bufs=2, space="PSUM")
        bcpool = _bc_ctx.__enter__()

        def bcast(dst, src_row):
            X = src_row.shape[-1]
            assert dst.shape[0] == 128 and dst.shape[-1] == X
            for lo in range(0, X, 512):
                w = min(512, X - lo)
                ps = bcpool.tile([128, 512], F32, tag="bc", name="bc_ps")
                te.matmul(ps[:, 0:w], ones_row[:], src_row[:, lo:lo + w],
                          start=True, stop=True)
                v.tensor_copy(dst[:, lo:lo + w], ps[:, 0:w])

        sy.dma_start(tmp_r[:], io["b_in"][None, :])
        bcast(bin_b, tmp_r[:])
        sy.dma_start(tmp_r[:], io["W_pool"][:])
        bcast(wpool_b, tmp_r[:])
        sy.dma_start(tmp_1[:], io["b_pool"][None, :])
        bcast(bpb, tmp_1[:])

        # ---- input projection h = nf @ W_in^T + b_in
        with tc.tile_pool(name="tp0", bufs=2, space="PSUM") as tpool, \
             tc.tile_pool(name="pp0", bufs=2, space="PSUM") as ppool, \
             tc.tile_pool(name="nf0", bufs=2) as nfpool:
            win_sb = nfpool.tile([128, 2, DIN], F32, tag="win", bufs=1)
            sy.dma_start(win_sb[:], io["W_in"].rearrange("(a p) d -> p a d", p=128))
            for kd in range(4):
                for ko in range(2):
                    pt = tpool.tile([128, 128], F32, tag="pt")
                    te.transpose(pt[:], win_sb[:, ko, kd * 128:(kd + 1) * 128],
                                 ident[:])
                    v.tensor_copy(winT[:, kd, ko * 128:(ko + 1) * 128], pt[:])
            for m in range(NBLK):
                nf_m = nfpool.tile([128, DIN], F32, tag="nf")
                sy.dma_start(nf_m[:], io["nf"][m * 128:(m + 1) * 128, :])
                nfT_m = nfpool.tile([128, 4, 128], F32, tag="nft")
                for kd in range(4):
                    pt = tpool.tile([128, 128], F32, tag="pt")
                    te.transpose(pt[:], nf_m[:, kd * 128:(kd + 1) * 128], ident[:])
                    v.tensor_copy(nfT_m[:, kd, :], pt[:])
                ph = ppool.tile([128, H], F32, tag="ph")
                for kd in range(4):
                    te.matmul(ph[:], nfT_m[:, kd, :], winT[:, kd, :],
                              start=(kd == 0), stop=(kd == 3))
                v.tensor_tensor(h_all[:, m, :], ph[:], bin_b[:], op=ALU.add)

        # ---- layers
        for l in range(L):
            bounce1 = dram.tile([NL, H], F8, name=f"bounce1_{l}")
            table = dram.tile([N, H], F8, name=f"table_{l}")
            bounce2 = dram.tile([3, 1024], F32, name=f"bounce2_{l}")
            ag2out = dram.tile([24, 1024], F32, addr_space="Shared",
                               name=f"ag2out_{l}")
            sy.dma_start(wg_sb[:], io["W_gat"][l].rearrange("(a p) d -> p a d",
                                                            p=128))
            sy.dma_start(a_cols[:], io["a_gat"][l].rearrange("(x p) -> p x", p=128))
            sy.dma_start(tmp_r[:], io["ln_g"][l][None, :])
            bcast(g_b, tmp_r[:])
            sy.dma_start(tmp_r[:], io["ln_b"][l][None, :])
            bcast(b_b, tmp_r[:])

            with tc.tile_pool(name=f"tp{l}", bufs=2, space="PSUM") as tpool, \
                 tc.tile_pool(name=f"pp{l}", bufs=3, space="PSUM") as ppool:
                for m in range(NBLK):
                    for kf in range(2):
                        pt = tpool.tile([128, 128], F32, tag="pt")
                        te.transpose(pt[:], h_all[:, m, kf * 128:(kf + 1) * 128],
                                     ident[:])
                        v.tensor_copy(ht[:, kf, m * 128:(m + 1) * 128], pt[:])
                for kf in range(2):
                    for ko in range(2):
                        pt = tpool.tile([128, 128], F32, tag="pt")
                        te.transpose(pt[:], wg_sb[:, ko, kf * 128:(kf + 1) * 128],
                                     ident[:])
                        v.tensor_copy(wgt_sb[:, kf, ko * 128:(ko + 1) * 128], pt[:])
                for it in range(2):
                    p1 = ppool.tile([128, 1], F32, tag="pp")
                    for ko in range(2):
                        te.matmul(p1[:], wg_sb[:, ko, it * 128:(it + 1) * 128],
                                  a_cols[:, ko:ko + 1], start=(ko == 0),
                                  stop=(ko == 1))
                    v.tensor_copy(a1p[:, it:it + 1], p1[:])
                    p2 = ppool.tile([128, 1], F32, tag="pp")
                    for ko in range(2):
                        te.matmul(p2[:], wg_sb[:, ko, it * 128:(it + 1) * 128],
                                  a_cols[:, 2 + ko:3 + ko], start=(ko == 0),
                                  stop=(ko == 1))
                    v.tensor_copy(a2p[:, it:it + 1], p2[:])
                for kf in range(2):
                    v.tensor_copy(rhs_l[:, kf, 0:H], wgt_sb[:, kf, :])
                    v.tensor_copy(rhs_l[:, kf, H:H + 1], a1p[:, kf:kf + 1])
                    v.tensor_copy(rhs_l[:, kf, H + 1:H + 2], a2p[:, kf:kf + 1])
                for m in range(NBLK):
                    ph = ppool.tile([128, H + 2], F32, tag="pp")
                    for kf in range(2):
                        te.matmul(ph[:], ht[:, kf, m * 128:(m + 1) * 128],
                                  rhs_l[:, kf, :], start=(kf == 0), stop=(kf == 1))
                    v.tensor_copy(hwpq[:, m, :], ph[:])
                    v.tensor_copy(tblin[:, m, :], ph[:, 0:H])
                v.memset(srow_t[:], 0.0)
                for half in range(2):
                    pp1 = ppool.tile([1, 512], F32, tag="pp")
                    for kf in range(2):
                        te.matmul(pp1[:], a1p[:, kf:kf + 1],
                                  ht[:, kf, half * 512:(half + 1) * 512],
                                  start=(kf == 0), stop=(kf == 1))
                    v.tensor_copy(prow_t[:, half * 512:(half + 1) * 512], pp1[:])
                    pq1 = ppool.tile([1, 512], F32, tag="pp")
                    for kf in range(2):
                        te.matmul(pq1[:], a2p[:, kf:kf + 1],
                                  ht[:, kf, half * 512:(half + 1) * 512],
                                  start=(kf == 0), stop=(kf == 1))
                    v.tensor_copy(qrow_t[:, half * 512:(half + 1) * 512], pq1[:])
                ps = ppool.tile([1, H + 2], F32, tag="pp")
                for m in range(NBLK):
                    te.matmul(ps[:], ones128[:], hwpq[:, m, :],
                              start=(m == 0), stop=(m == NBLK - 1))
                v.tensor_copy(srow_t[:, 0:H + 2], ps[:])

                sy.dma_start(bounce2[0:1, :], prow_t[:])
                sy.dma_start(bounce2[1:2, :], qrow_t[:])
                sy.dma_start(bounce2[2:3, :], srow_t[:])
                gp.collective_compute("AllGather", ALU.bypass, replica_groups=rg,
                                      ins=[bounce2[:].opt()], outs=[ag2out[:].opt()])
                sy.dma_start(bounce1[:].rearrange("(a p) d -> p a d", p=128),
                             tblin[:])
                gp.collective_compute("AllGather", ALU.bypass, replica_groups=rg,
                                      ins=[bounce1[:].opt()], outs=[table[:].opt()])

                ag2v = ag2out[:].rearrange("(r x) d -> r x d", x=3)
                sy.dma_start(stt[:], ag2v[:, 2, 0:H + 2])
                pst = ppool.tile([1, H + 2], F32, tag="pp")
                te.matmul(pst[:], ones8[:], stt[:], start=True, stop=True)
                v.tensor_copy(srow[:], pst[:])
                bcast(s_bcast, srow[:, 0:H])

                # q table: stage 8 q-rows into partition 0 then broadcast
                src_q = bass.AP(ag2out[:].tensor, 1024,
                                [[0, 1], [3072, 8], [1, 1024]])
                sy.dma_start(q_bcast[0:1, :].rearrange("p (r d) -> p r d", r=8),
                             src_q)
                bcast(q_bcast, q_bcast[0:1, :])
                bcast(p_bcast, prow_t[:])

                # per-edge coefficients, two half-stream passes
                for hf in range(2):
                    i0 = hf * nhalf
                    idx_sl = slice(hf * (ncht // 2), (hf + 1) * (ncht // 2))
                    gp.ap_gather(cstr_a[:], p_bcast[:], pidx_sb[:, idx_sl],
                                 128, NL, 1, nhalf)
                    gp.ap_gather(cstr_b[:], q_bcast[:], qidx_sb[:, idx_sl],
                                 128, N, 1, nhalf)
                    v.tensor_tensor(cstr_a[:], cstr_a[:], cstr_b[:], op=ALU.add)
                    v.tensor_scalar(cstr_b[:], cstr_a[:], 0.2, None, op0=ALU.mult)
                    v.tensor_tensor(cstr_a[:], cstr_a[:], cstr_b[:], op=ALU.max)
                    sc.activation(cstr_a[:], cstr_a[:], AF.Exp)
                    v.tensor_scalar(cstr_a[:], cstr_a[:], -1.0, None, op0=ALU.add)
                    v.tensor_tensor(cstr_a[:], cstr_a[:],
                                    pmask_sb[:, i0:i0 + nhalf], op=ALU.mult)
                    sy.dma_start(
                        bridge[:].rearrange("(p i) -> p i", p=128)[:, i0:i0 + nhalf],
                        cstr_a[:])
                # regroup: flat pos (16g+r)*nstream + r*ncht + col -> [p, col]
                src_ap = bass.AP(bridge[:].tensor, 0,
                                 [[16 * nstream, 8], [nstream + ncht, 16],
                                  [1, ncht]])
                sy.dma_start(c_all[:], src_ap)
                v.tensor_copy(c8[:], c_all[:])

            with tc.tile_pool(name=f"sp{l}", bufs=3, space="PSUM") as spool, \
                 tc.tile_pool(name=f"gp{l}", bufs=4) as gpool, \
                 tc.tile_pool(name=f"ep{l}", bufs=3) as ep:
                nreg = {}
                for k0 in range(0, ncb, 8):
                    kw = min(8, ncb - k0)
                    if kw * 128 not in nreg:
                        nreg[kw * 128] = gp.to_reg(kw * 128)
                for b in range(NBLK):
                    pb = spool.tile([128, H + 1], F32, tag="pb")
                    for k0 in range(0, ncb, 8):
                        kw = min(8, ncb - k0)
                        col = b * ncb + k0
                        g1 = gpool.tile([128, 8, H], F8, tag="g1")
                        gp.dma_gather(
                            g1[:, 0:kw, :], table[:],
                            gidx_sb[:, col * 8:(col + kw) * 8],
                            kw * 128, nreg[kw * 128], H)
                        wt = gpool.tile([128, 8, H + 1], F8, tag="wt")
                        csl = c8[:, col:col + kw]
                        v.tensor_tensor(wt[:, 0:kw, 0:H], g1[:, 0:kw, :],
                                        csl.unsqueeze(2).to_broadcast(
                                            [128, kw, H]), op=ALU.mult)
                        v.tensor_copy(wt[:, 0:kw, H], csl)
                        for k in range(kw):
                            te.matmul(pb[:], ind_sb[:, col + k, :],
                                      wt[:, k, :], start=(k0 + k == 0),
                                      stop=(k0 + k == ncb - 1))
                    denom = ep.tile([128, 1], F32, tag="den")
                    v.tensor_scalar(denom[:], pb[:, H:H + 1], float(N), None,
                                    op0=ALU.add)
                    v.reciprocal(denom[:], denom[:])
                    hn = ep.tile([128, H], F32, tag="hn")
                    v.tensor_tensor(hn[:], pb[:, 0:H], s_bcast[:], op=ALU.add)
                    v.tensor_scalar(hn[:], hn[:], denom[:], None, op0=ALU.mult)
                    v.tensor_tensor(hn[:], hn[:], h_all[:, b, :], op=ALU.add)
                    mu = ep.tile([128, 1], F32, tag="mu")
                    v.tensor_reduce(mu[:], hn[:], axis=mybir.AxisListType.X,
                                    op=ALU.add)
                    v.tensor_scalar(mu[:], mu[:], 1.0 / H, None, op0=ALU.mult)
                    v.tensor_scalar(hn[:], hn[:], mu[:], None, op0=ALU.subtract)
                    var = ep.tile([128, 1], F32, tag="var")
                    sq = ep.tile([128, H], F32, tag="sq")
                    sc.activation(sq[:], hn[:], AF.Square, accum_out=var[:])
                    v.tensor_scalar(var[:], var[:], 1.0 / H, None, op0=ALU.mult)
                    v.tensor_scalar(var[:], var[:], LN_EPS, None, op0=ALU.add)
                    sc.activation(var[:], var[:], AF.Sqrt)
                    v.reciprocal(var[:], var[:])
                    v.tensor_scalar(hn[:], hn[:], var[:], None, op0=ALU.mult)
                    v.tensor_tensor(hn[:], hn[:], g_b[:], op=ALU.mult)
                    v.tensor_tensor(hn[:], hn[:], b_b[:], op=ALU.add)
                    ex = ep.tile([128, H], F32, tag="ex")
                    sc.activation(ex[:], hn[:], AF.Exp)
                    v.tensor_scalar(ex[:], ex[:], -1.0, None, op0=ALU.add)
                    v.tensor_scalar(ex[:], ex[:], 0.0, None, op0=ALU.min)
                    v.tensor_tensor(h_all[:, b, :], hn[:], ex[:], op=ALU.max)

        # ---- readout
        with tc.tile_pool(name="fp", bufs=1, space="PSUM") as fpool, \
             tc.tile_pool(name="fe", bufs=3) as fe:
            pg = fpool.tile([1, H], F32, tag="pg")
            for b in range(NBLK):
                gl = fe.tile([128, H], F32, tag="gl")
                v.tensor_tensor(gl[:], h_all[:, b, :], wpool_b[:], op=ALU.mult)
                glr = fe.tile([128, 1], F32, tag="glr")
                v.tensor_reduce(glr[:], gl[:], axis=mybir.AxisListType.X,
                                op=ALU.add)
                v.tensor_tensor(glr[:], glr[:], bpb[:], op=ALU.add)
                sc.activation(glr[:], glr[:], AF.Sigmoid)
                gh = fe.tile([128, H], F32, tag="gh")
                v.tensor_scalar(gh[:], h_all[:, b, :], glr[:], None, op0=ALU.mult)
                te.matmul(pg[:], ones128[:], gh[:], start=(b == 0),
                          stop=(b == NBLK - 1))
            ge_sb = fe.tile([1, H], F32, tag="ge")
            v.tensor_copy(ge_sb[:], pg[:])
            sy.dma_start(io["ge_out"][:], ge_sb[:])
            sy.dma_start(io["h_out"].rearrange("(a p) d -> p a d", p=128), h_all[:])


def _declare_io(nc, ncb):
    ncht = NBLK * ncb
    io = {}

    def inp(name, shape, dt):
        io[name] = nc.dram_tensor(name, shape, dt, kind="ExternalInput").ap()

    inp("nf", [NL, DIN], F32)
    inp("W_in", [H, DIN], F32)
    inp("b_in", [H], F32)
    inp("W_gat", [L, H, H], F32)
    inp("a_gat", [L, 2 * H], F32)
    inp("ln_g", [L, H], F32)
    inp("ln_b", [L, H], F32)
    inp("W_pool", [1, H], F32)
    inp("b_pool", [1], F32)
    inp("ident", [128, 128], F32)
    inp("gidx", [128, ncht * 8], I16)
    inp("pidx", [128, ncht], I16)
    inp("qidx", [128, ncht], I16)
    inp("pmask", [128, 16 * ncht], F8)
    inp("ind", [128, ncht * 128], F8)
    io["h_out"] = nc.dram_tensor("h_out", [NL, H], F32, kind="ExternalOutput").ap()
    io["ge_out"] = nc.dram_tensor("ge_out", [1, H], F32, kind="ExternalOutput").ap()
    return io


def build(ncb):
    nc = bacc.Bacc("TRN2", target_bir_lowering=False, debug=False,
                   num_devices=NCORES)
    io = _declare_io(nc, ncb)
    with tile.TileContext(nc) as tc:
        _emit(nc, tc, ncb, io)
    nc.compile()
    return nc


def make_in_maps(inputs):
    per_core, ncb = _preprocess(inputs["edge_index"])
    nf = np.asarray(inputs["node_features"], np.float32)
    base = {k: np.ascontiguousarray(np.asarray(inputs[k], np.float32))
            for k in ("W_in", "b_in", "W_gat", "a_gat", "ln_g", "ln_b",
                      "W_pool", "b_pool")}
    base["ident"] = np.eye(128, dtype=np.float32)
    in_maps = []
    for c in range(NCORES):
        gidx, ind, pmask, pidx, qidx = _build_core_arrays(per_core[c], ncb)
        m = dict(base)
        m["nf"] = np.ascontiguousarray(nf[c * NL:(c + 1) * NL])
        m["gidx"], m["ind"], m["pmask"] = gidx, ind, pmask
        m["pidx"], m["qidx"] = pidx, qidx
        in_maps.append(m)
    return in_maps, ncb


def kernel(**inputs):
    from concourse.bass_utils import run_bass_kernel_spmd
    in_maps, ncb = make_in_maps(inputs)
    nc = build(ncb)
    res = run_bass_kernel_spmd(nc, in_maps, list(range(NCORES)))
    h = np.concatenate([r["h_out"] for r in res.results], axis=0)
    ge = np.stack([r["ge_out"][0] for r in res.results]).sum(0).astype(np.float32)
    return h, ge
